# revision 22
# baseline (speedup 1.0000x reference)
"""DGM network (nn_DGMNetT) Trainium2 Bass kernel.

Math (reference):
    X  = [t, x]                       # [N, 2]
    S1 = tanh(X @ W0 + b0)            # [N, 256]
    per layer l (3 layers):
        Z  = tanh(X @ Uz + S  @ Wz + bz)
        G  = tanh(X @ Ug + S1 @ Wg + bg)
        R  = tanh(X @ Ur + S  @ Wr + br)
        Hg = tanh(X @ Uh + (S*R) @ Wh + bh)
        S  = (1-G)*Hg + Z*S
    out = S @ Wf + bf                 # [N, 1]

Kernel strategy (pure data parallel over N across 8 cores; everything in
TRANSPOSED layout so X^T rows are just t / x / ones vectors):

  * fp16 operands, fp32 PSUM accumulation (fp8 DoubleRow was measured at
    4-5x over the 2e-2 max-err budget, so the 256-wide contraction runs as
    2 fp16 k-chunk matmuls with the k-loop outermost so each stationary
    load serves 4 tiles).
  * Halved-state trick: layers keep St = S/2 so (1-G)/2 = sigmoid(-2*u_g)
    =: C gives  St' = C*Hg + Z*St.  Host pre-scales W' = 2W for layers >=
    1 (C gate and layer 0 consume the UNhalved S1; layer 0 folds the /2
    into p2 via scalar_tensor_tensor(z, 0.5, s1)).
  * Biases folded into the X-side matmuls via a ones row (K=3 stationary).
  * Quad processing (4 row-tiles of 512), m-split: a gate-step covers
    (gate, m-half) x 4 tiles in one 4-bank PSUM tile [128, 4, 512]; the
    4 X-matmuls go to row groups 32*tau (concurrent), the 8 W-matmuls
    share 2 stationary loads; ONE [128, 2048] activation per step (the
    352-cycle ScalarE overhead amortizes 2x vs per-tile acts).
  * ScalarE is the roofline: 13 tanh/sigmoid passes over every element
    (26 x ~2us activations per quad).  PE/DVE are scheduled to hide
    under it; PSUM ping-pongs 2 x 4-bank step tiles.
  * Emission is phase-major over groups of 2 quads, gate order C,R,Z,H
    (C first: it needs only S1, so the new state's fp8 copy has two
    activation slots of slack at each layer boundary).  Next group's
    init is interleaved before the current group's final projection.
  * Redundant InstLdweights (k-outer runs reload the same stationary
    once per matmul) are deduped post-schedule; HW-measured timing-
    neutral (the PE reorder window hides them) but trims ~1200
    instructions.  The final projection is k-outer for the same reason.

Measured on HW (rep-loop slope): ~550 us/core steady state.  Ablations
put the PE path at ~527 us (binding) and the ACT path at ~423 us.
Microbenchmarks show sustained full-width fp16 matmuls stream at
~0.52 ns/col (~1.92 GHz effective, likely P0 power state), so the
W-side runs at the silicon's achievable rate; 32-col stationaries hit
the nominal 2.4 GHz but do 4x less work per column.  Variants measured
and rejected: finer PSUM steps (2-bank ring-4: +15%), split
activations (neutral), palindrome bank order (neutral), bf16
(identical), GQ=1 (+1.5%), fp8 DoubleRow (needs fp8 moving state:
accuracy-fatal), multi-bank matmul out (ISA-illegal).
"""

import numpy as np

N_TOTAL = 131072
N_CORES = 8
R_CORE = N_TOTAL // N_CORES  # 16384 rows per core
TN = 512                     # rows per tile (matmul moving free dim)
T_TILES = R_CORE // TN       # 32 tiles per core
QUADS = T_TILES // 4         # 8 quads per core
H = 256
L = 3

_CACHE = {}


def _build_program(repeat=None):
    # repeat: perf-measurement only — wraps the whole tile loop in an
    # on-device For_i so (wall(K) - wall(1))/(K-1) isolates device time
    # from the dispatch overhead. kernel() always uses repeat=None.
    from contextlib import ExitStack


    import concourse.bacc as bacc
    import concourse.mybir as mybir
    import concourse.tile as tile

    f16 = mybir.dt.float16
    f32 = mybir.dt.float32
    AF = mybir.ActivationFunctionType

    nc = bacc.Bacc("TRN2", target_bir_lowering=False)

    x3_d = nc.dram_tensor("X3", [T_TILES, 3, TN], f16, kind="ExternalInput")
    ua_d = nc.dram_tensor("UA", [L, 4, 128, 256], f16, kind="ExternalInput")
    w16_d = nc.dram_tensor("W16", [L, 4, 2, 2, 128, 128], f16, kind="ExternalInput")
    w0_d = nc.dram_tensor("W0R", [128, 256], f16, kind="ExternalInput")
    wf_d = nc.dram_tensor("WF2", [128, 2], f16, kind="ExternalInput")
    out_d = nc.dram_tensor("out", [T_TILES, TN], f32, kind="ExternalOutput")

    GQ = 2  # quads per pipeline group (8 tiles)

    with tile.TileContext(nc) as tc, ExitStack() as ctx:
        wpool = ctx.enter_context(tc.tile_pool(name="wpool", bufs=1))
        xtp = ctx.enter_context(tc.tile_pool(name="xtp", bufs=2 * GQ))
        s1p = ctx.enter_context(tc.tile_pool(name="s1p", bufs=1))
        stp = ctx.enter_context(tc.tile_pool(name="stp", bufs=1))
        gp = ctx.enter_context(tc.tile_pool(name="gp", bufs=1))
        ps = ctx.enter_context(tc.tile_pool(name="ps", bufs=2, space="PSUM"))

        def ps_pair(name):
            # two independently-semaphored 2-bank halves instead of one
            # 4-bank tile: act(A) starts mid-fill and frees its banks one
            # act-duration earlier, keeping the ring fill-bound
            pa = ps.tile([128, 2, TN], f32, tag="psA", name=name + "a")
            pb = ps.tile([128, 2, TN], f32, tag="psB", name=name + "b")
            return pa, pb

        # ---- resident weights (w0 first: the very first init matmuls
        # need only w0 + xt) ----
        w0_sb = wpool.tile([128, 256], f16, tag="w0", name="w0_sb")
        nc.sync.dma_start(out=w0_sb[:], in_=w0_d[:])
        wf_sb = wpool.tile([128, 2], f16, tag="wfp", name="wf_sb")
        nc.sync.dma_start(out=wf_sb[:], in_=wf_d[:])
        # ua/w loads go on the gpsimd queue (layer 0 first) so the sync
        # queue serves only w0 + the xt tiles the init matmuls need first
        ua_sb = {}
        w_sb = {}
        for l in range(L):
            for g in range(4):
                u = wpool.tile([128, 256], f16, tag=f"ua{l}{g}", name=f"ua{l}{g}")
                nc.gpsimd.dma_start(out=u[:], in_=ua_d[l, g])
                ua_sb[(l, g)] = u
                for m in range(2):
                    w = wpool.tile(
                        [128, 2, 128], f16, tag=f"w{l}{g}{m}", name=f"w{l}{g}{m}"
                    )
                    nc.gpsimd.dma_start(
                        out=w[:, 0, :], in_=w16_d[l, g, m, 0]
                    )
                    nc.gpsimd.dma_start(
                        out=w[:, 1, :], in_=w16_d[l, g, m, 1]
                    )
                    w_sb[(l, g, m)] = w

        # gate order in weight packing: 0=C(G), 1=Z, 2=R, 3=Hg
        def x_mms(psg, src, m, xt, t0, start=True, stop=False):
            # X-side matmuls (K=3, carries the bias via the ones row),
            # one per tile tau at row group 32*tau -> concurrent on HW.
            for j, t in enumerate(range(t0, t0 + 2)):
                nc.tensor.matmul(
                    out=psg[:, j, :],
                    lhsT=src[32 * t : 32 * t + 3, m * 128 : (m + 1) * 128],
                    rhs=xt[32 * t : 32 * t + 3, :],
                    start=start,
                    stop=stop,
                    tile_position=(32 * t, 0),
                )

        def w_mms(psg, l, g, m, s, t0):
            # k-loop outermost: each stationary k-chunk load serves the
            # half-step's 2 back-to-back matmuls
            for k in range(2):
                for j, t in enumerate(range(t0, t0 + 2)):
                    nc.tensor.matmul(
                        out=psg[:, j, :],
                        lhsT=w_sb[(l, g, m)][:, k, :],
                        rhs=s[:, k, t, :],
                        start=False,
                        stop=(k == 1),
                        tile_position=(0, 0),
                    )

        def gate_mms(pab, l, g, m, v, skey):
            # one gate-step: X-side (K=3) + W-side (K=256), halves A then B
            for h, psg in enumerate(pab):
                x_mms(psg, ua_sb[(l, g)], m, v["xt"], t0=2 * h)
                w_mms(psg, l, g, m, v[skey], t0=2 * h)

        def emit_init(quads):
            st = {}
            for q in quads:
                xt = xtp.tile([128, TN], f16, tag="xt", name=f"xt{q}")
                for t in range(4):
                    nc.sync.dma_start(
                        out=xt[32 * t : 32 * t + 3, :], in_=x3_d[4 * q + t]
                    )
                s1_16 = s1p.tile([128, 2, 4, TN], f16, tag="s1", bufs=GQ,
                                 name=f"s1_{q}")
                for m in range(2):
                    pab = ps_pair(f"psi{q}_{m}")
                    for h, psi in enumerate(pab):
                        x_mms(psi, w0_sb, m, xt, t0=2 * h, start=True,
                              stop=True)
                        nc.scalar.activation(
                            out=s1_16[:, m, 2 * h : 2 * h + 2, :],
                            in_=psi[:], func=AF.Tanh
                        )
                st[q] = dict(xt=xt, s1_16=s1_16, s16=s1_16)
            return st

        def emit_layers(st):
            quads = list(st)
            for l in range(L):
                # ---- C gate (rhs = S1 always; no new-state dependency) ----
                for q in quads:
                    v = st[q]
                    c16 = gp.tile([128, 2, 4, TN], f16, tag="c", bufs=GQ + 1,
                                  name=f"c{q}_{l}")
                    for m in range(2):
                        pab = ps_pair(f"psc{q}_{l}_{m}")
                        gate_mms(pab, l, 0, m, v, "s1_16")
                        for h, psc in enumerate(pab):
                            nc.scalar.activation(
                                out=c16[:, m, 2 * h : 2 * h + 2, :],
                                in_=psc[:], func=AF.Sigmoid, scale=-2.0
                            )
                    v["c16"] = c16
                # ---- R gate, then S*R (fp8) while Z runs ----
                for q in quads:
                    v = st[q]
                    r16 = gp.tile([128, 2, 4, TN], f16, tag="r", bufs=2,
                                  name=f"r{q}_{l}")
                    for m in range(2):
                        pab = ps_pair(f"psr{q}_{l}_{m}")
                        gate_mms(pab, l, 2, m, v, "s16")
                        for h, psr in enumerate(pab):
                            nc.scalar.activation(
                                out=r16[:, m, 2 * h : 2 * h + 2, :],
                                in_=psr[:], func=AF.Tanh
                            )
                    sr16 = gp.tile([128, 2, 4, TN], f16, tag="sr", bufs=2,
                                   name=f"sr{q}_{l}")
                    for m in range(2):
                        nc.vector.tensor_mul(sr16[:, m], v["s16"][:, m], r16[:, m])
                    v["sr16"] = sr16
                # ---- Z gate, then p2 = Z*S ----
                for q in quads:
                    v = st[q]
                    z16 = gp.tile([128, 2, 4, TN], f16, tag="z", bufs=2,
                                  name=f"z{q}_{l}")
                    for m in range(2):
                        pab = ps_pair(f"psz{q}_{l}_{m}")
                        gate_mms(pab, l, 1, m, v, "s16")
                        for h, psz in enumerate(pab):
                            nc.scalar.activation(
                                out=z16[:, m, 2 * h : 2 * h + 2, :],
                                in_=psz[:], func=AF.Tanh
                            )
                    p2 = gp.tile([128, 2, 4, TN], f16, tag="p2", bufs=GQ,
                                 name=f"p2_{q}_{l}")
                    for m in range(2):
                        if l == 0:
                            # layer-0 state is the UNhalved S1; fold the /2 here
                            nc.vector.scalar_tensor_tensor(
                                p2[:, m], z16[:, m], 0.5, v["s16"][:, m],
                                op0=mybir.AluOpType.mult, op1=mybir.AluOpType.mult,
                            )
                        else:
                            nc.vector.tensor_mul(p2[:, m], z16[:, m], v["s16"][:, m])
                    v["p2"] = p2
                # ---- Hg gate + state update ----
                for q in quads:
                    v = st[q]
                    h16 = gp.tile([128, 2, 4, TN], f16, tag="h", bufs=2,
                                  name=f"h{q}_{l}")
                    for m in range(2):
                        pab = ps_pair(f"psh{q}_{l}_{m}")
                        gate_mms(pab, l, 3, m, v, "sr16")
                        for h, psh in enumerate(pab):
                            nc.scalar.activation(
                                out=h16[:, m, 2 * h : 2 * h + 2, :],
                                in_=psh[:], func=AF.Tanh
                            )
                    p1 = gp.tile([128, 2, 4, TN], f16, tag="p1", bufs=2,
                                 name=f"p1_{q}_{l}")
                    for m in range(2):
                        nc.vector.tensor_mul(p1[:, m], v["c16"][:, m], h16[:, m])
                    s16n = stp.tile([128, 2, 4, TN], f16, tag="s16n", bufs=GQ + 1,
                                    name=f"s16_{q}_{l}")
                    for m in range(2):
                        nc.vector.tensor_add(s16n[:, m], p1[:, m], v["p2"][:, m])
                    v["s16"] = s16n

        def emit_final(st):
            # out = St3 @ (2*Wf); bf is added on the host
            for q in st:
                v = st[q]
                ocp = gp.tile([1, 4, TN], f32, tag="ocp", bufs=2, name=f"ocp{q}")
                fab = ps_pair(f"fp{q}")
                for h, fp in enumerate(fab):
                    # k-outer so the two wf stationary loads serve 2 tiles
                    for k in range(2):
                        for j, t in enumerate(range(2 * h, 2 * h + 2)):
                            nc.tensor.matmul(
                                out=fp[0:1, j, :],
                                lhsT=wf_sb[:, k : k + 1],
                                rhs=v["s16"][:, k, t, :],
                                start=(k == 0),
                                stop=(k == 1),
                                tile_position=(0, 0),
                            )
                    nc.vector.tensor_copy(
                        ocp[:, 2 * h : 2 * h + 2, :], fp[0:1, :, :]
                    )
                # gpsimd queue: on the sync queue this would head-of-line
                # block the next group's xt loads (queues execute in order)
                nc.gpsimd.dma_start(
                    out=out_d[4 * q : 4 * q + 4, :], in_=ocp[0:1, :, :]
                )

        def emit_tiles():
            groups = [range(g0, g0 + GQ) for g0 in range(0, QUADS, GQ)]
            # interleave: next group's init phase is emitted before this
            # group's final phase, so PE has init matmuls to chew on while
            # the last layer's activations drain
            st = emit_init(groups[0])
            for gi in range(len(groups)):
                emit_layers(st)
                nxt = emit_init(groups[gi + 1]) if gi + 1 < len(groups) else None
                emit_final(st)
                st = nxt

        if repeat is not None:
            with tc.For_i(0, repeat, 1):
                emit_tiles()
        else:
            emit_tiles()

    _dedupe_ldweights(nc, mybir)
    nc.compile()
    return nc


def _dedupe_ldweights(nc, mybir):
    """Drop InstLdweights that reload the stationary already in the PE array.

    The k-outer matmul loops emit runs of matmuls sharing one stationary, but
    bass pairs every InstMatmult with its own InstLdweights; on HW each reload
    streams 128 columns (~107 ns) and cannot overlap an in-flight matmul on
    the same row groups (measured ~112 us/core exposed).  A later LDW is
    redundant iff it has the same weights AP/tile placement as the previous
    one and every PE-array instruction in between is a plain matmul.  Any
    sync_info on the dropped LDW is merged into the next PE instruction.
    """
    n_drop = 0
    for blk in nc.main_func.blocks:
        keep = []
        last_sig = None
        pending_sync = None

        def _merge_into(inst):
            nonlocal pending_sync
            if pending_sync is None:
                return
            si = inst.sync_info
            if si is None:
                inst.sync_info = pending_sync
            else:
                si.on_wait.extend(pending_sync.on_wait)
                si.on_update.extend(pending_sync.on_update)
            pending_sync = None

        for inst in blk.instructions:
            tn = type(inst).__name__
            if tn == "InstLdweights":
                sig = (
                    str(inst.ins[0]),
                    inst.tile_position,
                    inst.tile_size,
                    inst.perf_mode,
                    inst.is_transpose,
                )
                if sig == last_sig:
                    # redundant reload: drop, carry its sync forward
                    if inst.sync_info is not None and (
                        inst.sync_info.on_wait or inst.sync_info.on_update
                    ):
                        if pending_sync is None:
                            pending_sync = inst.sync_info
                        else:
                            pending_sync.on_wait.extend(inst.sync_info.on_wait)
                            pending_sync.on_update.extend(
                                inst.sync_info.on_update
                            )
                    n_drop += 1
                    continue
                last_sig = sig
                keep.append(inst)
                continue
            if getattr(inst, "engine", None) == mybir.EngineType.PE:
                if tn == "InstMatmult" and not inst.is_transpose:
                    _merge_into(inst)
                else:
                    # anything else touching the PE array invalidates reuse
                    _merge_into(inst)
                    if tn != "InstEventSemaphore":
                        last_sig = None
            keep.append(inst)
        assert pending_sync is None, "dropped LDW sync had no PE successor"
        if len(keep) != len(blk.instructions):
            del blk.instructions[:]
            blk.instructions.extend(keep)
    return n_drop


def _pack_weights(inp):
    f16 = np.float16
    Ws = {}
    # gate order 0=C(G),1=Z,2=R,3=Hg
    u_keys = ["Ug", "Uz", "Ur", "Uh"]
    b_keys = ["bg", "bz", "br", "bh"]
    w_keys = ["Wg", "Wz", "Wr", "Wh"]

    # X-side stationaries, replicated across all 4 row groups so the 4
    # tiles of a quad can run their K=3 matmuls concurrently
    ua = np.zeros((L, 4, 128, 256), f16)
    for l in range(L):
        for g in range(4):
            for rg in range(4):
                ua[l, g, 32 * rg + 0] = inp[u_keys[g]][l][0].astype(f16)
                ua[l, g, 32 * rg + 1] = inp[u_keys[g]][l][1].astype(f16)
                ua[l, g, 32 * rg + 2] = inp[b_keys[g]][l][0].astype(f16)
    Ws["UA"] = ua

    # k-chunk stationaries: [l, g, m, k, ki, mp] = W'[128*k+ki, 128*m+mp]
    w16 = np.zeros((L, 4, 2, 2, 128, 128), f16)
    for l in range(L):
        for g in range(4):
            # C's W consumes the UNhalved S1 always; layer 0's Z/R/H consume
            # the unhalved S1 too -> no 2x there; halved states elsewhere
            scale = 1.0 if (g == 0 or l == 0) else 2.0
            wl = (inp[w_keys[g]][l] * scale).astype(np.float32)  # [256, 256]
            for m in range(2):
                for k in range(2):
                    w16[l, g, m, k] = wl[
                        128 * k : 128 * (k + 1), 128 * m : 128 * (m + 1)
                    ].astype(f16)
    Ws["W16"] = w16

    w0r = np.zeros((128, 256), f16)
    for rg in range(4):
        w0r[32 * rg + 0] = inp["W0"][0].astype(f16)
        w0r[32 * rg + 1] = inp["W0"][1].astype(f16)
        w0r[32 * rg + 2] = inp["b0"][0].astype(f16)
    Ws["W0R"] = w0r

    wf2 = np.zeros((128, 2), f16)
    for k in range(2):
        wf2[:, k] = (inp["Wf"][128 * k : 128 * (k + 1), 0] * 2.0).astype(f16)
    Ws["WF2"] = wf2
    return Ws


def _make_core_inputs(t_core, x_core, Ws):
    x3 = np.empty((T_TILES, 3, TN), np.float16)
    x3[:, 0, :] = t_core.reshape(T_TILES, TN)
    x3[:, 1, :] = x_core.reshape(T_TILES, TN)
    x3[:, 2, :] = 1.0
    return dict(X3=x3, **Ws)


def kernel(**inputs):
    from concourse import bass_utils

    inp = {k: np.asarray(v) for k, v in inputs.items()}

    if "nc" not in _CACHE:
        _CACHE["nc"] = _build_program()
    nc = _CACHE["nc"]

    Ws = _pack_weights(inp)

    t_all = inp["t"].reshape(N_CORES, R_CORE).astype(np.float16)
    x_all = inp["x"].reshape(N_CORES, R_CORE).astype(np.float16)
    in_maps = []
    for c in range(N_CORES):
        in_maps.append(_make_core_inputs(t_all[c], x_all[c], Ws))

    res = bass_utils.run_bass_kernel_spmd(nc, in_maps, core_ids=list(range(N_CORES)))
    out = np.concatenate([r["out"].reshape(-1) for r in res.results])
    out = out.reshape(N_TOTAL, 1) + inp["bf"].astype(np.float32)
    return out.astype(np.float32)



# revision 23
# speedup vs baseline: 1.1964x; 1.1964x over previous
"""DGM network (nn_DGMNetT) Trainium2 Bass kernel.

Math (reference):
    X  = [t, x]                       # [N, 2]
    S1 = tanh(X @ W0 + b0)            # [N, 256]
    per layer l (3 layers):
        Z  = tanh(X @ Uz + S  @ Wz + bz)
        G  = tanh(X @ Ug + S1 @ Wg + bg)
        R  = tanh(X @ Ur + S  @ Wr + br)
        Hg = tanh(X @ Uh + (S*R) @ Wh + bh)
        S  = (1-G)*Hg + Z*S
    out = S @ Wf + bf                 # [N, 1]

Kernel strategy (pure data parallel over N across 8 cores; everything in
TRANSPOSED layout so X^T rows are just t / x / ones vectors):

  * fp16 operands, fp32 PSUM accumulation (fp8 DoubleRow was measured at
    4-5x over the 2e-2 max-err budget, so the 256-wide contraction runs as
    2 fp16 k-chunk matmuls with the k-loop outermost so each stationary
    load serves 4 tiles).
  * Halved-state trick: layers keep St = S/2 so (1-G)/2 = sigmoid(-2*u_g)
    =: C gives  St' = C*Hg + Z*St.  Host pre-scales W' = 2W for layers >=
    1 (C gate and layer 0 consume the UNhalved S1; layer 0 folds the /2
    into p2 via scalar_tensor_tensor(z, 0.5, s1)).
  * Biases folded into the X-side matmuls via a ones row (K=3 stationary).
  * Quad processing (4 row-tiles of 512), m-split: a gate-step covers
    (gate, m-half) x 4 tiles in one 4-bank PSUM tile [128, 4, 512]; the
    4 X-matmuls go to row groups 32*tau (concurrent), the 8 W-matmuls
    share 2 stationary loads; ONE [128, 2048] activation per step (the
    352-cycle ScalarE overhead amortizes 2x vs per-tile acts).
  * ScalarE is the roofline: 13 tanh/sigmoid passes over every element
    (26 x ~2us activations per quad).  PE/DVE are scheduled to hide
    under it; PSUM ping-pongs 2 x 4-bank step tiles.
  * Emission is phase-major over groups of 2 quads, gate order C,R,Z,H
    (C first: it needs only S1, so the new state's fp8 copy has two
    activation slots of slack at each layer boundary).  Next group's
    init is interleaved before the current group's final projection.
  * Redundant InstLdweights (k-outer runs reload the same stationary
    once per matmul) are deduped post-schedule; HW-measured timing-
    neutral (the PE reorder window hides them) but trims ~1200
    instructions.  The final projection is k-outer for the same reason.

Measured on HW (rep-loop slope): ~550 us/core steady state.  Ablations
put the PE path at ~527 us (binding) and the ACT path at ~423 us.
Microbenchmarks show sustained full-width fp16 matmuls stream at
~0.52 ns/col (~1.92 GHz effective, likely P0 power state), so the
W-side runs at the silicon's achievable rate; 32-col stationaries hit
the nominal 2.4 GHz but do 4x less work per column.  Variants measured
and rejected: finer PSUM steps (2-bank ring-4: +15%), split
activations (neutral), palindrome bank order (neutral), bf16
(identical), GQ=1 (+1.5%), fp8 DoubleRow (needs fp8 moving state:
accuracy-fatal), multi-bank matmul out (ISA-illegal).
"""

import numpy as np

N_TOTAL = 131072
N_CORES = 8
R_CORE = N_TOTAL // N_CORES  # 16384 rows per core
TN = 512                     # rows per tile (matmul moving free dim)
T_TILES = R_CORE // TN       # 32 tiles per core
QUADS = T_TILES // 4         # 8 quads per core
H = 256
L = 3

_CACHE = {}


def _build_program(repeat=None):
    # repeat: perf-measurement only — wraps the whole tile loop in an
    # on-device For_i so (wall(K) - wall(1))/(K-1) isolates device time
    # from the dispatch overhead. kernel() always uses repeat=None.
    from contextlib import ExitStack


    import concourse.bacc as bacc
    import concourse.mybir as mybir
    import concourse.tile as tile

    f16 = mybir.dt.float16
    f32 = mybir.dt.float32
    AF = mybir.ActivationFunctionType

    nc = bacc.Bacc("TRN2", target_bir_lowering=False)

    x3_d = nc.dram_tensor("X3", [T_TILES, 3, TN], f16, kind="ExternalInput")
    ua_d = nc.dram_tensor("UA", [L, 4, 128, 256], f16, kind="ExternalInput")
    w16_d = nc.dram_tensor("W16", [L, 4, 2, 2, 128, 128], f16, kind="ExternalInput")
    w0_d = nc.dram_tensor("W0R", [128, 256], f16, kind="ExternalInput")
    wf_d = nc.dram_tensor("WF2", [128, 2], f16, kind="ExternalInput")
    out_d = nc.dram_tensor("out", [T_TILES, TN], f32, kind="ExternalOutput")

    GQ = 2  # quads per pipeline group (8 tiles)

    with tile.TileContext(nc) as tc, ExitStack() as ctx:
        wpool = ctx.enter_context(tc.tile_pool(name="wpool", bufs=1))
        xtp = ctx.enter_context(tc.tile_pool(name="xtp", bufs=2 * GQ))
        s1p = ctx.enter_context(tc.tile_pool(name="s1p", bufs=1))
        stp = ctx.enter_context(tc.tile_pool(name="stp", bufs=1))
        gp = ctx.enter_context(tc.tile_pool(name="gp", bufs=1))
        ps = ctx.enter_context(tc.tile_pool(name="ps", bufs=2, space="PSUM"))

        def ps_pair(name):
            # two independently-semaphored 2-bank halves instead of one
            # 4-bank tile: act(A) starts mid-fill and frees its banks one
            # act-duration earlier, keeping the ring fill-bound
            pa = ps.tile([128, 2, TN], f32, tag="psA", name=name + "a")
            pb = ps.tile([128, 2, TN], f32, tag="psB", name=name + "b")
            return pa, pb

        # ---- resident weights (w0 first: the very first init matmuls
        # need only w0 + xt) ----
        w0_sb = wpool.tile([128, 256], f16, tag="w0", name="w0_sb")
        nc.sync.dma_start(out=w0_sb[:], in_=w0_d[:])
        wf_sb = wpool.tile([128, 2], f16, tag="wfp", name="wf_sb")
        nc.sync.dma_start(out=wf_sb[:], in_=wf_d[:])
        # ua/w loads go on the gpsimd queue (layer 0 first) so the sync
        # queue serves only w0 + the xt tiles the init matmuls need first
        ua_sb = {}
        w_sb = {}
        for l in range(L):
            for g in range(4):
                u = wpool.tile([128, 256], f16, tag=f"ua{l}{g}", name=f"ua{l}{g}")
                nc.gpsimd.dma_start(out=u[:], in_=ua_d[l, g])
                ua_sb[(l, g)] = u
                for m in range(2):
                    w = wpool.tile(
                        [128, 2, 128], f16, tag=f"w{l}{g}{m}", name=f"w{l}{g}{m}"
                    )
                    nc.gpsimd.dma_start(
                        out=w[:, 0, :], in_=w16_d[l, g, m, 0]
                    )
                    nc.gpsimd.dma_start(
                        out=w[:, 1, :], in_=w16_d[l, g, m, 1]
                    )
                    w_sb[(l, g, m)] = w

        # gate order in weight packing: 0=C(G), 1=Z, 2=R, 3=Hg
        def x_mms(psg, src, m, xt, t0, start=True, stop=False):
            # X-side matmuls (K=3, carries the bias via the ones row),
            # one per tile tau at row group 32*tau -> concurrent on HW.
            for j, t in enumerate(range(t0, t0 + 2)):
                nc.tensor.matmul(
                    out=psg[:, j, :],
                    lhsT=src[32 * t : 32 * t + 3, m * 128 : (m + 1) * 128],
                    rhs=xt[32 * t : 32 * t + 3, :],
                    start=start,
                    stop=stop,
                    tile_position=(32 * t, 0),
                )

        def w_mms(psg, l, g, m, s, t0):
            # k-loop outermost: each stationary k-chunk load serves the
            # half-step's 2 back-to-back matmuls
            for k in range(2):
                for j, t in enumerate(range(t0, t0 + 2)):
                    nc.tensor.matmul(
                        out=psg[:, j, :],
                        lhsT=w_sb[(l, g, m)][:, k, :],
                        rhs=s[:, k, t, :],
                        start=False,
                        stop=(k == 1),
                        tile_position=(0, 0),
                    )

        def gate_mms(pab, l, g, m, v, skey, act):
            # one gate-step into two half-tiles: one adjacent 4-way X burst
            # (keeps row-group concurrency), then W(A), act(A), W(B), act(B)
            # so act(A) runs while the B half fills and frees its banks one
            # act-duration earlier
            for h, psg in enumerate(pab):
                x_mms(psg, ua_sb[(l, g)], m, v["xt"], t0=2 * h)
            for h, psg in enumerate(pab):
                w_mms(psg, l, g, m, v[skey], t0=2 * h)
                act(h, psg)

        def emit_init(quads):
            st = {}
            for q in quads:
                xt = xtp.tile([128, TN], f16, tag="xt", name=f"xt{q}")
                for t in range(4):
                    nc.sync.dma_start(
                        out=xt[32 * t : 32 * t + 3, :], in_=x3_d[4 * q + t]
                    )
                s1_16 = s1p.tile([128, 2, 4, TN], f16, tag="s1", bufs=GQ,
                                 name=f"s1_{q}")
                for m in range(2):
                    pab = ps_pair(f"psi{q}_{m}")
                    for h, psi in enumerate(pab):
                        x_mms(psi, w0_sb, m, xt, t0=2 * h, start=True,
                              stop=True)
                    for h, psi in enumerate(pab):
                        nc.scalar.activation(
                            out=s1_16[:, m, 2 * h : 2 * h + 2, :],
                            in_=psi[:], func=AF.Tanh
                        )
                st[q] = dict(xt=xt, s1_16=s1_16, s16=s1_16)
            return st

        def emit_layers(st):
            quads = list(st)
            for l in range(L):
                # ---- C gate (rhs = S1 always; no new-state dependency) ----
                for q in quads:
                    v = st[q]
                    c16 = gp.tile([128, 2, 4, TN], f16, tag="c", bufs=GQ + 1,
                                  name=f"c{q}_{l}")
                    for m in range(2):
                        pab = ps_pair(f"psc{q}_{l}_{m}")

                        def _act_c(h, psg, m=m, c16=c16):
                            nc.scalar.activation(
                                out=c16[:, m, 2 * h : 2 * h + 2, :],
                                in_=psg[:], func=AF.Sigmoid, scale=-2.0
                            )

                        gate_mms(pab, l, 0, m, v, "s1_16", _act_c)
                    v["c16"] = c16
                # ---- R gate, then S*R (fp8) while Z runs ----
                for q in quads:
                    v = st[q]
                    r16 = gp.tile([128, 2, 4, TN], f16, tag="r", bufs=2,
                                  name=f"r{q}_{l}")
                    for m in range(2):
                        pab = ps_pair(f"psr{q}_{l}_{m}")

                        def _act(h, psg, m=m, _t=r16):
                            nc.scalar.activation(
                                out=_t[:, m, 2 * h : 2 * h + 2, :],
                                in_=psg[:], func=AF.Tanh
                            )

                        gate_mms(pab, l, 2, m, v, "s16", _act)
                    sr16 = gp.tile([128, 2, 4, TN], f16, tag="sr", bufs=2,
                                   name=f"sr{q}_{l}")
                    for m in range(2):
                        nc.vector.tensor_mul(sr16[:, m], v["s16"][:, m], r16[:, m])
                    v["sr16"] = sr16
                # ---- Z gate, then p2 = Z*S ----
                for q in quads:
                    v = st[q]
                    z16 = gp.tile([128, 2, 4, TN], f16, tag="z", bufs=2,
                                  name=f"z{q}_{l}")
                    for m in range(2):
                        pab = ps_pair(f"psz{q}_{l}_{m}")

                        def _act(h, psg, m=m, _t=z16):
                            nc.scalar.activation(
                                out=_t[:, m, 2 * h : 2 * h + 2, :],
                                in_=psg[:], func=AF.Tanh
                            )

                        gate_mms(pab, l, 1, m, v, "s16", _act)
                    p2 = gp.tile([128, 2, 4, TN], f16, tag="p2", bufs=GQ,
                                 name=f"p2_{q}_{l}")
                    for m in range(2):
                        if l == 0:
                            # layer-0 state is the UNhalved S1; fold the /2 here
                            nc.vector.scalar_tensor_tensor(
                                p2[:, m], z16[:, m], 0.5, v["s16"][:, m],
                                op0=mybir.AluOpType.mult, op1=mybir.AluOpType.mult,
                            )
                        else:
                            nc.vector.tensor_mul(p2[:, m], z16[:, m], v["s16"][:, m])
                    v["p2"] = p2
                # ---- Hg gate + state update ----
                for q in quads:
                    v = st[q]
                    h16 = gp.tile([128, 2, 4, TN], f16, tag="h", bufs=2,
                                  name=f"h{q}_{l}")
                    for m in range(2):
                        pab = ps_pair(f"psh{q}_{l}_{m}")

                        def _act(h, psg, m=m, _t=h16):
                            nc.scalar.activation(
                                out=_t[:, m, 2 * h : 2 * h + 2, :],
                                in_=psg[:], func=AF.Tanh
                            )

                        gate_mms(pab, l, 3, m, v, "sr16", _act)
                    p1 = gp.tile([128, 2, 4, TN], f16, tag="p1", bufs=2,
                                 name=f"p1_{q}_{l}")
                    for m in range(2):
                        nc.vector.tensor_mul(p1[:, m], v["c16"][:, m], h16[:, m])
                    s16n = stp.tile([128, 2, 4, TN], f16, tag="s16n", bufs=GQ + 1,
                                    name=f"s16_{q}_{l}")
                    for m in range(2):
                        nc.vector.tensor_add(s16n[:, m], p1[:, m], v["p2"][:, m])
                    v["s16"] = s16n

        def emit_final(st):
            # out = St3 @ (2*Wf); bf is added on the host
            for q in st:
                v = st[q]
                ocp = gp.tile([1, 4, TN], f32, tag="ocp", bufs=2, name=f"ocp{q}")
                fab = ps_pair(f"fp{q}")
                for h, fp in enumerate(fab):
                    # k-outer so the two wf stationary loads serve 2 tiles
                    for k in range(2):
                        for j, t in enumerate(range(2 * h, 2 * h + 2)):
                            nc.tensor.matmul(
                                out=fp[0:1, j, :],
                                lhsT=wf_sb[:, k : k + 1],
                                rhs=v["s16"][:, k, t, :],
                                start=(k == 0),
                                stop=(k == 1),
                                tile_position=(0, 0),
                            )
                    nc.vector.tensor_copy(
                        ocp[:, 2 * h : 2 * h + 2, :], fp[0:1, :, :]
                    )
                # gpsimd queue: on the sync queue this would head-of-line
                # block the next group's xt loads (queues execute in order)
                nc.gpsimd.dma_start(
                    out=out_d[4 * q : 4 * q + 4, :], in_=ocp[0:1, :, :]
                )

        def emit_tiles():
            groups = [range(g0, g0 + GQ) for g0 in range(0, QUADS, GQ)]
            # interleave: next group's init phase is emitted before this
            # group's final phase, so PE has init matmuls to chew on while
            # the last layer's activations drain
            st = emit_init(groups[0])
            for gi in range(len(groups)):
                emit_layers(st)
                nxt = emit_init(groups[gi + 1]) if gi + 1 < len(groups) else None
                emit_final(st)
                st = nxt

        if repeat is not None:
            with tc.For_i(0, repeat, 1):
                emit_tiles()
        else:
            emit_tiles()

    _dedupe_ldweights(nc, mybir)
    nc.compile()
    return nc


def _dedupe_ldweights(nc, mybir):
    """Drop InstLdweights that reload the stationary already in the PE array.

    The k-outer matmul loops emit runs of matmuls sharing one stationary, but
    bass pairs every InstMatmult with its own InstLdweights; on HW each reload
    streams 128 columns (~107 ns) and cannot overlap an in-flight matmul on
    the same row groups (measured ~112 us/core exposed).  A later LDW is
    redundant iff it has the same weights AP/tile placement as the previous
    one and every PE-array instruction in between is a plain matmul.  Any
    sync_info on the dropped LDW is merged into the next PE instruction.
    """
    n_drop = 0
    for blk in nc.main_func.blocks:
        keep = []
        last_sig = None
        pending_sync = None

        def _merge_into(inst):
            nonlocal pending_sync
            if pending_sync is None:
                return
            si = inst.sync_info
            if si is None:
                inst.sync_info = pending_sync
            else:
                si.on_wait.extend(pending_sync.on_wait)
                si.on_update.extend(pending_sync.on_update)
            pending_sync = None

        for inst in blk.instructions:
            tn = type(inst).__name__
            if tn == "InstLdweights":
                sig = (
                    str(inst.ins[0]),
                    inst.tile_position,
                    inst.tile_size,
                    inst.perf_mode,
                    inst.is_transpose,
                )
                if sig == last_sig:
                    # redundant reload: drop, carry its sync forward
                    if inst.sync_info is not None and (
                        inst.sync_info.on_wait or inst.sync_info.on_update
                    ):
                        if pending_sync is None:
                            pending_sync = inst.sync_info
                        else:
                            pending_sync.on_wait.extend(inst.sync_info.on_wait)
                            pending_sync.on_update.extend(
                                inst.sync_info.on_update
                            )
                    n_drop += 1
                    continue
                last_sig = sig
                keep.append(inst)
                continue
            if getattr(inst, "engine", None) == mybir.EngineType.PE:
                if tn == "InstMatmult" and not inst.is_transpose:
                    _merge_into(inst)
                else:
                    # anything else touching the PE array invalidates reuse
                    _merge_into(inst)
                    if tn != "InstEventSemaphore":
                        last_sig = None
            keep.append(inst)
        assert pending_sync is None, "dropped LDW sync had no PE successor"
        if len(keep) != len(blk.instructions):
            del blk.instructions[:]
            blk.instructions.extend(keep)
    return n_drop


def _pack_weights(inp):
    f16 = np.float16
    Ws = {}
    # gate order 0=C(G),1=Z,2=R,3=Hg
    u_keys = ["Ug", "Uz", "Ur", "Uh"]
    b_keys = ["bg", "bz", "br", "bh"]
    w_keys = ["Wg", "Wz", "Wr", "Wh"]

    # X-side stationaries, replicated across all 4 row groups so the 4
    # tiles of a quad can run their K=3 matmuls concurrently
    ua = np.zeros((L, 4, 128, 256), f16)
    for l in range(L):
        for g in range(4):
            for rg in range(4):
                ua[l, g, 32 * rg + 0] = inp[u_keys[g]][l][0].astype(f16)
                ua[l, g, 32 * rg + 1] = inp[u_keys[g]][l][1].astype(f16)
                ua[l, g, 32 * rg + 2] = inp[b_keys[g]][l][0].astype(f16)
    Ws["UA"] = ua

    # k-chunk stationaries: [l, g, m, k, ki, mp] = W'[128*k+ki, 128*m+mp]
    w16 = np.zeros((L, 4, 2, 2, 128, 128), f16)
    for l in range(L):
        for g in range(4):
            # C's W consumes the UNhalved S1 always; layer 0's Z/R/H consume
            # the unhalved S1 too -> no 2x there; halved states elsewhere
            scale = 1.0 if (g == 0 or l == 0) else 2.0
            wl = (inp[w_keys[g]][l] * scale).astype(np.float32)  # [256, 256]
            for m in range(2):
                for k in range(2):
                    w16[l, g, m, k] = wl[
                        128 * k : 128 * (k + 1), 128 * m : 128 * (m + 1)
                    ].astype(f16)
    Ws["W16"] = w16

    w0r = np.zeros((128, 256), f16)
    for rg in range(4):
        w0r[32 * rg + 0] = inp["W0"][0].astype(f16)
        w0r[32 * rg + 1] = inp["W0"][1].astype(f16)
        w0r[32 * rg + 2] = inp["b0"][0].astype(f16)
    Ws["W0R"] = w0r

    wf2 = np.zeros((128, 2), f16)
    for k in range(2):
        wf2[:, k] = (inp["Wf"][128 * k : 128 * (k + 1), 0] * 2.0).astype(f16)
    Ws["WF2"] = wf2
    return Ws


def _make_core_inputs(t_core, x_core, Ws):
    x3 = np.empty((T_TILES, 3, TN), np.float16)
    x3[:, 0, :] = t_core.reshape(T_TILES, TN)
    x3[:, 1, :] = x_core.reshape(T_TILES, TN)
    x3[:, 2, :] = 1.0
    return dict(X3=x3, **Ws)


def kernel(**inputs):
    from concourse import bass_utils

    inp = {k: np.asarray(v) for k, v in inputs.items()}

    if "nc" not in _CACHE:
        _CACHE["nc"] = _build_program()
    nc = _CACHE["nc"]

    Ws = _pack_weights(inp)

    t_all = inp["t"].reshape(N_CORES, R_CORE).astype(np.float16)
    x_all = inp["x"].reshape(N_CORES, R_CORE).astype(np.float16)
    in_maps = []
    for c in range(N_CORES):
        in_maps.append(_make_core_inputs(t_all[c], x_all[c], Ws))

    res = bass_utils.run_bass_kernel_spmd(nc, in_maps, core_ids=list(range(N_CORES)))
    out = np.concatenate([r["out"].reshape(-1) for r in res.results])
    out = out.reshape(N_TOTAL, 1) + inp["bf"].astype(np.float32)
    return out.astype(np.float32)



# revision 24
# speedup vs baseline: 1.2201x; 1.0199x over previous
"""DGM network (nn_DGMNetT) Trainium2 Bass kernel.

Math (reference):
    X  = [t, x]                       # [N, 2]
    S1 = tanh(X @ W0 + b0)            # [N, 256]
    per layer l (3 layers):
        Z  = tanh(X @ Uz + S  @ Wz + bz)
        G  = tanh(X @ Ug + S1 @ Wg + bg)
        R  = tanh(X @ Ur + S  @ Wr + br)
        Hg = tanh(X @ Uh + (S*R) @ Wh + bh)
        S  = (1-G)*Hg + Z*S
    out = S @ Wf + bf                 # [N, 1]

Kernel strategy (pure data parallel over N across 8 cores; everything in
TRANSPOSED layout so X^T rows are just t / x / ones vectors):

  * fp16 operands, fp32 PSUM accumulation (fp8 DoubleRow was measured at
    4-5x over the 2e-2 max-err budget, so the 256-wide contraction runs as
    2 fp16 k-chunk matmuls with the k-loop outermost so each stationary
    load serves 4 tiles).
  * Halved-state trick: layers keep St = S/2 so (1-G)/2 = sigmoid(-2*u_g)
    =: C gives  St' = C*Hg + Z*St.  Host pre-scales W' = 2W for layers >=
    1 (C gate and layer 0 consume the UNhalved S1; layer 0 folds the /2
    into p2 via scalar_tensor_tensor(z, 0.5, s1)).
  * Biases folded into the X-side matmuls via a ones row (K=3 stationary).
  * Quad processing (4 row-tiles of 512), m-split: a gate-step covers
    (gate, m-half) x 4 tiles in one 4-bank PSUM tile [128, 4, 512]; the
    4 X-matmuls go to row groups 32*tau (concurrent), the 8 W-matmuls
    share 2 stationary loads; ONE [128, 2048] activation per step (the
    352-cycle ScalarE overhead amortizes 2x vs per-tile acts).
  * ScalarE is the roofline: 13 tanh/sigmoid passes over every element
    (26 x ~2us activations per quad).  PE/DVE are scheduled to hide
    under it; PSUM ping-pongs 2 x 4-bank step tiles.
  * Emission is phase-major over groups of 2 quads, gate order C,R,Z,H
    (C first: it needs only S1, so the new state's fp8 copy has two
    activation slots of slack at each layer boundary).  Next group's
    init is interleaved before the current group's final projection.
  * Redundant InstLdweights (k-outer runs reload the same stationary
    once per matmul) are deduped post-schedule; HW-measured timing-
    neutral (the PE reorder window hides them) but trims ~1200
    instructions.  The final projection is k-outer for the same reason.

Measured on HW (rep-loop slope): ~550 us/core steady state.  Ablations
put the PE path at ~527 us (binding) and the ACT path at ~423 us.
Microbenchmarks show sustained full-width fp16 matmuls stream at
~0.52 ns/col (~1.92 GHz effective, likely P0 power state), so the
W-side runs at the silicon's achievable rate; 32-col stationaries hit
the nominal 2.4 GHz but do 4x less work per column.  Variants measured
and rejected: finer PSUM steps (2-bank ring-4: +15%), split
activations (neutral), palindrome bank order (neutral), bf16
(identical), GQ=1 (+1.5%), fp8 DoubleRow (needs fp8 moving state:
accuracy-fatal), multi-bank matmul out (ISA-illegal).
"""

import numpy as np

N_TOTAL = 131072
N_CORES = 8
R_CORE = N_TOTAL // N_CORES  # 16384 rows per core
TN = 512                     # rows per tile (matmul moving free dim)
T_TILES = R_CORE // TN       # 32 tiles per core
QUADS = T_TILES // 4         # 8 quads per core
H = 256
L = 3

_CACHE = {}


def _build_program(repeat=None):
    # repeat: perf-measurement only — wraps the whole tile loop in an
    # on-device For_i so (wall(K) - wall(1))/(K-1) isolates device time
    # from the dispatch overhead. kernel() always uses repeat=None.
    from contextlib import ExitStack


    import concourse.bacc as bacc
    import concourse.mybir as mybir
    import concourse.tile as tile

    f16 = mybir.dt.float16
    f32 = mybir.dt.float32
    AF = mybir.ActivationFunctionType

    nc = bacc.Bacc("TRN2", target_bir_lowering=False)

    x3_d = nc.dram_tensor("X3", [T_TILES, 3, TN], f16, kind="ExternalInput")
    ua_d = nc.dram_tensor("UA", [L, 4, 128, 256], f16, kind="ExternalInput")
    w16_d = nc.dram_tensor("W16", [L, 4, 2, 2, 128, 128], f16, kind="ExternalInput")
    w0_d = nc.dram_tensor("W0R", [128, 256], f16, kind="ExternalInput")
    wf_d = nc.dram_tensor("WF2", [128, 2], f16, kind="ExternalInput")
    out_d = nc.dram_tensor("out", [T_TILES, TN], f32, kind="ExternalOutput")

    GQ = 2  # quads per pipeline group (8 tiles)

    with tile.TileContext(nc) as tc, ExitStack() as ctx:
        wpool = ctx.enter_context(tc.tile_pool(name="wpool", bufs=1))
        xtp = ctx.enter_context(tc.tile_pool(name="xtp", bufs=2 * GQ))
        s1p = ctx.enter_context(tc.tile_pool(name="s1p", bufs=1))
        stp = ctx.enter_context(tc.tile_pool(name="stp", bufs=1))
        gp = ctx.enter_context(tc.tile_pool(name="gp", bufs=1))
        ps = ctx.enter_context(tc.tile_pool(name="ps", bufs=2, space="PSUM"))

        def ps_pair(name):
            # asymmetric 3+1 split with separate tags (= separate slot
            # semaphores): the X burst stays 4-way row-group concurrent,
            # act(A) covers tiles 0-2 starting at ~75% of the fill, and
            # act(B) (one tile, ~570 ns) frees its bank right after the
            # fill, so neither slot gates the next-next step's X burst
            pa = ps.tile([128, 3, TN], f32, tag="psA", name=name + "a")
            pb = ps.tile([128, 1, TN], f32, tag="psB", name=name + "b")
            return pa, pb

        HALF_TS = ((0, 1, 2), (3,))

        # ---- resident weights (w0 first: the very first init matmuls
        # need only w0 + xt) ----
        w0_sb = wpool.tile([128, 256], f16, tag="w0", name="w0_sb")
        nc.sync.dma_start(out=w0_sb[:], in_=w0_d[:])
        wf_sb = wpool.tile([128, 2], f16, tag="wfp", name="wf_sb")
        nc.sync.dma_start(out=wf_sb[:], in_=wf_d[:])
        # ua/w loads go on the gpsimd queue (layer 0 first) so the sync
        # queue serves only w0 + the xt tiles the init matmuls need first
        ua_sb = {}
        w_sb = {}
        for l in range(L):
            for g in range(4):
                u = wpool.tile([128, 256], f16, tag=f"ua{l}{g}", name=f"ua{l}{g}")
                nc.gpsimd.dma_start(out=u[:], in_=ua_d[l, g])
                ua_sb[(l, g)] = u
                for m in range(2):
                    w = wpool.tile(
                        [128, 2, 128], f16, tag=f"w{l}{g}{m}", name=f"w{l}{g}{m}"
                    )
                    nc.gpsimd.dma_start(
                        out=w[:, 0, :], in_=w16_d[l, g, m, 0]
                    )
                    nc.gpsimd.dma_start(
                        out=w[:, 1, :], in_=w16_d[l, g, m, 1]
                    )
                    w_sb[(l, g, m)] = w

        # gate order in weight packing: 0=C(G), 1=Z, 2=R, 3=Hg
        def x_mms(psg, src, m, xt, ts, start=True, stop=False):
            # X-side matmuls (K=3, carries the bias via the ones row),
            # one per tile tau at row group 32*tau -> concurrent on HW.
            for j, t in enumerate(ts):
                nc.tensor.matmul(
                    out=psg[:, j, :],
                    lhsT=src[32 * t : 32 * t + 3, m * 128 : (m + 1) * 128],
                    rhs=xt[32 * t : 32 * t + 3, :],
                    start=start,
                    stop=stop,
                    tile_position=(32 * t, 0),
                )

        def w_mms(psg, l, g, m, s, ts):
            # k-loop outermost: each stationary k-chunk load serves the
            # half-step's back-to-back matmuls
            for k in range(2):
                for j, t in enumerate(ts):
                    nc.tensor.matmul(
                        out=psg[:, j, :],
                        lhsT=w_sb[(l, g, m)][:, k, :],
                        rhs=s[:, k, t, :],
                        start=False,
                        stop=(k == 1),
                        tile_position=(0, 0),
                    )

        def gate_mms(pab, l, g, m, v, skey, act):
            # one gate-step into two half-tiles: one adjacent 4-way X burst
            # (keeps row-group concurrency), then W(A), act(A), W(B), act(B)
            # so act(A) runs while the B half fills and frees its banks one
            # act-duration earlier
            for h, psg in enumerate(pab):
                x_mms(psg, ua_sb[(l, g)], m, v["xt"], ts=HALF_TS[h])
            for h, psg in enumerate(pab):
                w_mms(psg, l, g, m, v[skey], ts=HALF_TS[h])
                act(h, psg)

        def emit_init(quads):
            st = {}
            for q in quads:
                xt = xtp.tile([128, TN], f16, tag="xt", name=f"xt{q}")
                for t in range(4):
                    nc.sync.dma_start(
                        out=xt[32 * t : 32 * t + 3, :], in_=x3_d[4 * q + t]
                    )
                s1_16 = s1p.tile([128, 2, 4, TN], f16, tag="s1", bufs=GQ,
                                 name=f"s1_{q}")
                for m in range(2):
                    pab = ps_pair(f"psi{q}_{m}")
                    for h, psi in enumerate(pab):
                        x_mms(psi, w0_sb, m, xt, ts=HALF_TS[h], start=True,
                              stop=True)
                    for h, psi in enumerate(pab):
                        nc.scalar.activation(
                            out=s1_16[:, m, 3 * h : 3 + h, :],
                            in_=psi[:], func=AF.Tanh
                        )
                st[q] = dict(xt=xt, s1_16=s1_16, s16=s1_16)
            return st

        def emit_layers(st):
            quads = list(st)
            for l in range(L):
                # ---- C gate (rhs = S1 always; no new-state dependency) ----
                for q in quads:
                    v = st[q]
                    c16 = gp.tile([128, 2, 4, TN], f16, tag="c", bufs=GQ + 1,
                                  name=f"c{q}_{l}")
                    for m in range(2):
                        pab = ps_pair(f"psc{q}_{l}_{m}")

                        def _act_c(h, psg, m=m, c16=c16):
                            nc.scalar.activation(
                                out=c16[:, m, 3 * h : 3 + h, :],
                                in_=psg[:], func=AF.Sigmoid, scale=-2.0
                            )

                        gate_mms(pab, l, 0, m, v, "s1_16", _act_c)
                    v["c16"] = c16
                # ---- R gate, then S*R (fp8) while Z runs ----
                for q in quads:
                    v = st[q]
                    r16 = gp.tile([128, 2, 4, TN], f16, tag="r", bufs=2,
                                  name=f"r{q}_{l}")
                    for m in range(2):
                        pab = ps_pair(f"psr{q}_{l}_{m}")

                        def _act(h, psg, m=m, _t=r16):
                            nc.scalar.activation(
                                out=_t[:, m, 3 * h : 3 + h, :],
                                in_=psg[:], func=AF.Tanh
                            )

                        gate_mms(pab, l, 2, m, v, "s16", _act)
                    sr16 = gp.tile([128, 2, 4, TN], f16, tag="sr", bufs=2,
                                   name=f"sr{q}_{l}")
                    for m in range(2):
                        nc.vector.tensor_mul(sr16[:, m], v["s16"][:, m], r16[:, m])
                    v["sr16"] = sr16
                # ---- Z gate, then p2 = Z*S ----
                for q in quads:
                    v = st[q]
                    z16 = gp.tile([128, 2, 4, TN], f16, tag="z", bufs=2,
                                  name=f"z{q}_{l}")
                    for m in range(2):
                        pab = ps_pair(f"psz{q}_{l}_{m}")

                        def _act(h, psg, m=m, _t=z16):
                            nc.scalar.activation(
                                out=_t[:, m, 3 * h : 3 + h, :],
                                in_=psg[:], func=AF.Tanh
                            )

                        gate_mms(pab, l, 1, m, v, "s16", _act)
                    p2 = gp.tile([128, 2, 4, TN], f16, tag="p2", bufs=GQ,
                                 name=f"p2_{q}_{l}")
                    for m in range(2):
                        if l == 0:
                            # layer-0 state is the UNhalved S1; fold the /2 here
                            nc.vector.scalar_tensor_tensor(
                                p2[:, m], z16[:, m], 0.5, v["s16"][:, m],
                                op0=mybir.AluOpType.mult, op1=mybir.AluOpType.mult,
                            )
                        else:
                            nc.vector.tensor_mul(p2[:, m], z16[:, m], v["s16"][:, m])
                    v["p2"] = p2
                # ---- Hg gate + state update ----
                for q in quads:
                    v = st[q]
                    h16 = gp.tile([128, 2, 4, TN], f16, tag="h", bufs=2,
                                  name=f"h{q}_{l}")
                    for m in range(2):
                        pab = ps_pair(f"psh{q}_{l}_{m}")

                        def _act(h, psg, m=m, _t=h16):
                            nc.scalar.activation(
                                out=_t[:, m, 3 * h : 3 + h, :],
                                in_=psg[:], func=AF.Tanh
                            )

                        gate_mms(pab, l, 3, m, v, "sr16", _act)
                    p1 = gp.tile([128, 2, 4, TN], f16, tag="p1", bufs=2,
                                 name=f"p1_{q}_{l}")
                    for m in range(2):
                        nc.vector.tensor_mul(p1[:, m], v["c16"][:, m], h16[:, m])
                    s16n = stp.tile([128, 2, 4, TN], f16, tag="s16n", bufs=GQ + 1,
                                    name=f"s16_{q}_{l}")
                    for m in range(2):
                        nc.vector.tensor_add(s16n[:, m], p1[:, m], v["p2"][:, m])
                    v["s16"] = s16n

        def emit_final(st):
            # out = St3 @ (2*Wf); bf is added on the host
            for q in st:
                v = st[q]
                ocp = gp.tile([1, 4, TN], f32, tag="ocp", bufs=2, name=f"ocp{q}")
                fab = ps_pair(f"fp{q}")
                for h, fp in enumerate(fab):
                    # k-outer so the two wf stationary loads serve the tiles
                    for k in range(2):
                        for j, t in enumerate(HALF_TS[h]):
                            nc.tensor.matmul(
                                out=fp[0:1, j, :],
                                lhsT=wf_sb[:, k : k + 1],
                                rhs=v["s16"][:, k, t, :],
                                start=(k == 0),
                                stop=(k == 1),
                                tile_position=(0, 0),
                            )
                    nc.vector.tensor_copy(
                        ocp[:, 3 * h : 3 + h, :], fp[0:1, :, :]
                    )
                # gpsimd queue: on the sync queue this would head-of-line
                # block the next group's xt loads (queues execute in order)
                nc.gpsimd.dma_start(
                    out=out_d[4 * q : 4 * q + 4, :], in_=ocp[0:1, :, :]
                )

        def emit_tiles():
            groups = [range(g0, g0 + GQ) for g0 in range(0, QUADS, GQ)]
            # interleave: next group's init phase is emitted before this
            # group's final phase, so PE has init matmuls to chew on while
            # the last layer's activations drain
            st = emit_init(groups[0])
            for gi in range(len(groups)):
                emit_layers(st)
                nxt = emit_init(groups[gi + 1]) if gi + 1 < len(groups) else None
                emit_final(st)
                st = nxt

        if repeat is not None:
            with tc.For_i(0, repeat, 1):
                emit_tiles()
        else:
            emit_tiles()

    _dedupe_ldweights(nc, mybir)
    nc.compile()
    return nc


def _dedupe_ldweights(nc, mybir):
    """Drop InstLdweights that reload the stationary already in the PE array.

    The k-outer matmul loops emit runs of matmuls sharing one stationary, but
    bass pairs every InstMatmult with its own InstLdweights; on HW each reload
    streams 128 columns (~107 ns) and cannot overlap an in-flight matmul on
    the same row groups (measured ~112 us/core exposed).  A later LDW is
    redundant iff it has the same weights AP/tile placement as the previous
    one and every PE-array instruction in between is a plain matmul.  Any
    sync_info on the dropped LDW is merged into the next PE instruction.
    """
    n_drop = 0
    for blk in nc.main_func.blocks:
        keep = []
        last_sig = None
        pending_sync = None

        def _merge_into(inst):
            nonlocal pending_sync
            if pending_sync is None:
                return
            si = inst.sync_info
            if si is None:
                inst.sync_info = pending_sync
            else:
                si.on_wait.extend(pending_sync.on_wait)
                si.on_update.extend(pending_sync.on_update)
            pending_sync = None

        for inst in blk.instructions:
            tn = type(inst).__name__
            if tn == "InstLdweights":
                sig = (
                    str(inst.ins[0]),
                    inst.tile_position,
                    inst.tile_size,
                    inst.perf_mode,
                    inst.is_transpose,
                )
                if sig == last_sig:
                    # redundant reload: drop, carry its sync forward
                    if inst.sync_info is not None and (
                        inst.sync_info.on_wait or inst.sync_info.on_update
                    ):
                        if pending_sync is None:
                            pending_sync = inst.sync_info
                        else:
                            pending_sync.on_wait.extend(inst.sync_info.on_wait)
                            pending_sync.on_update.extend(
                                inst.sync_info.on_update
                            )
                    n_drop += 1
                    continue
                last_sig = sig
                keep.append(inst)
                continue
            if getattr(inst, "engine", None) == mybir.EngineType.PE:
                if tn == "InstMatmult" and not inst.is_transpose:
                    _merge_into(inst)
                else:
                    # anything else touching the PE array invalidates reuse
                    _merge_into(inst)
                    if tn != "InstEventSemaphore":
                        last_sig = None
            keep.append(inst)
        assert pending_sync is None, "dropped LDW sync had no PE successor"
        if len(keep) != len(blk.instructions):
            del blk.instructions[:]
            blk.instructions.extend(keep)
    return n_drop


def _pack_weights(inp):
    f16 = np.float16
    Ws = {}
    # gate order 0=C(G),1=Z,2=R,3=Hg
    u_keys = ["Ug", "Uz", "Ur", "Uh"]
    b_keys = ["bg", "bz", "br", "bh"]
    w_keys = ["Wg", "Wz", "Wr", "Wh"]

    # X-side stationaries, replicated across all 4 row groups so the 4
    # tiles of a quad can run their K=3 matmuls concurrently
    ua = np.zeros((L, 4, 128, 256), f16)
    for l in range(L):
        for g in range(4):
            for rg in range(4):
                ua[l, g, 32 * rg + 0] = inp[u_keys[g]][l][0].astype(f16)
                ua[l, g, 32 * rg + 1] = inp[u_keys[g]][l][1].astype(f16)
                ua[l, g, 32 * rg + 2] = inp[b_keys[g]][l][0].astype(f16)
    Ws["UA"] = ua

    # k-chunk stationaries: [l, g, m, k, ki, mp] = W'[128*k+ki, 128*m+mp]
    w16 = np.zeros((L, 4, 2, 2, 128, 128), f16)
    for l in range(L):
        for g in range(4):
            # C's W consumes the UNhalved S1 always; layer 0's Z/R/H consume
            # the unhalved S1 too -> no 2x there; halved states elsewhere
            scale = 1.0 if (g == 0 or l == 0) else 2.0
            wl = (inp[w_keys[g]][l] * scale).astype(np.float32)  # [256, 256]
            for m in range(2):
                for k in range(2):
                    w16[l, g, m, k] = wl[
                        128 * k : 128 * (k + 1), 128 * m : 128 * (m + 1)
                    ].astype(f16)
    Ws["W16"] = w16

    w0r = np.zeros((128, 256), f16)
    for rg in range(4):
        w0r[32 * rg + 0] = inp["W0"][0].astype(f16)
        w0r[32 * rg + 1] = inp["W0"][1].astype(f16)
        w0r[32 * rg + 2] = inp["b0"][0].astype(f16)
    Ws["W0R"] = w0r

    wf2 = np.zeros((128, 2), f16)
    for k in range(2):
        wf2[:, k] = (inp["Wf"][128 * k : 128 * (k + 1), 0] * 2.0).astype(f16)
    Ws["WF2"] = wf2
    return Ws


def _make_core_inputs(t_core, x_core, Ws):
    x3 = np.empty((T_TILES, 3, TN), np.float16)
    x3[:, 0, :] = t_core.reshape(T_TILES, TN)
    x3[:, 1, :] = x_core.reshape(T_TILES, TN)
    x3[:, 2, :] = 1.0
    return dict(X3=x3, **Ws)


def kernel(**inputs):
    from concourse import bass_utils

    inp = {k: np.asarray(v) for k, v in inputs.items()}

    if "nc" not in _CACHE:
        _CACHE["nc"] = _build_program()
    nc = _CACHE["nc"]

    Ws = _pack_weights(inp)

    t_all = inp["t"].reshape(N_CORES, R_CORE).astype(np.float16)
    x_all = inp["x"].reshape(N_CORES, R_CORE).astype(np.float16)
    in_maps = []
    for c in range(N_CORES):
        in_maps.append(_make_core_inputs(t_all[c], x_all[c], Ws))

    res = bass_utils.run_bass_kernel_spmd(nc, in_maps, core_ids=list(range(N_CORES)))
    out = np.concatenate([r["out"].reshape(-1) for r in res.results])
    out = out.reshape(N_TOTAL, 1) + inp["bf"].astype(np.float32)
    return out.astype(np.float32)

